# revision 16
# baseline (speedup 1.0000x reference)
"""Low-rank Cayley linear kernel for TRN2 (8 NeuronCores, batch-sharded).

Math: reference computes W = (I+A) @ NS4(I-A) with A = U V^T - V U^T and
NS4 = 4 Newton-Schulz iterations for (I-A)^{-1} starting at X=I, which is
exactly the partial Neumann sum X4 = sum_{j=0}^{15} A^j.  With
C = [U, V] (n x 2r), D = [V, -U] (n x 2r) we have A = C D^T and
A^{j+1} = C E^j D^T where E = D^T C is (2r x 2r).  Therefore

    W = (I + A) X4 = I + C F D^T,   F = 2 * sum_{j=0}^{14} E^j + E^15

and the output is

    y = x @ W^T = x + (x @ D) @ (F^T C^T).

All the 2048^3 work collapses to two rank-128 GEMMs per token plus a
128x128 polynomial evaluated once (8x fewer FLOPs).

v16 (fp16 end to end, measured rel err ~1.2e-3, gate 2e-2):
  - x up / y down as fp16 (8 MB each per core); fp16 transposes + GEMM
    operands at the 1-cycle/row PE rate, fp32 PSUM accumulation.
  - Host marshals [C | D | C^T] into one partition-major fp16 tensor
    (fat descriptors; a row-major layout costs ~14 us of HWDGE
    descriptor generation per load and stalls the in-order PE queue).
  - The whole Newton-Schulz chain runs in fp16 (fp32 PSUM) off the
    uploaded C/D; S = F^T C^T uses the uploaded C^T directly.
  - 256-token tiles: per tile, PE-transpose 128x128 blocks -> x^T
    chunks (ACT copies 1024-wide out of PSUM), stage1 P^T = D^T x^T
    (16 accumulating fp16 matmuls), stage2 corr = P @ S as [128,1024]
    PSUM pairs, y = x + corr on DVE (1024-wide) writing fp16, 0.5 MB
    stores right behind each sub-block's add.
  - Emission order = engine queue order: x loads and the first tile's
    transposes are emitted BEFORE the chain so the in-order PE queue
    starts on transposes (x data lands ~8.5 us) instead of blocking on
    the weight load (~12.5 us).
  - All pools persist; main pools are created first so nothing aliases
    the streaming tiles (an alias adds a WAR edge that serializes the
    x loads behind the setup chain).
"""

import numpy as np

import concourse.bacc as bacc
import concourse.bass as bass
import concourse.mybir as mybir
import concourse.tile as tile
from concourse.bass_utils import run_bass_kernel_spmd
from concourse.masks import make_identity

N = 2048          # model dim (N_IN == N_OUT)
R = 64            # rank of U, V
R2 = 2 * R        # 128
NCORES = 8
TOK = 2048        # tokens per core (one batch element)
F32 = mybir.dt.float32
F16 = mybir.dt.float16
NCHUNK = N // 128          # 16 feature chunks
NTILE = 8                  # 256-token tiles
NSUB = 2                   # 128-token sub-blocks per tile
TTOK = NSUB * 128          # tokens per tile
HDT = np.float16

_NC_CACHE = {}


def _chain(nc, tc, w_d, const, setup, ps_s, identB, S_sb):
    """Emit weight construction (E powers -> F -> S).  Returns D_sb view."""
    # fused weight tile: [C | D | C^T] on the scalar HWDGE ring; C|D land
    # first (they gate the E accumulation), C^T arrives later
    wsb = const.tile([128, 3, N], F16)
    w_r = w_d[:].rearrange("p (t f) -> p t f", t=3)
    nc.scalar.dma_start(out=wsb[:, 0:2], in_=w_r[:, 0:2])
    nc.scalar.dma_start(out=wsb[:, 2], in_=w_r[:, 2])
    C_sb = wsb[:, 0].rearrange("p (j q) -> p j q", j=NCHUNK)
    D_sb = wsb[:, 1].rearrange("p (j q) -> p j q", j=NCHUNK)
    CT = wsb[:, 2]

    counter = [0]

    def fresh():
        counter[0] += 1
        return setup.tile([128, 128], F16, name=f"sm{counter[0]}", tag=f"sm{counter[0]}")

    def accum_mm(lhs_view, rhs_view):
        ps = ps_s.tile([128, 512], F32, tag="small_mm")
        for j in range(NCHUNK):
            nc.tensor.matmul(
                ps[:, 0:128],
                lhs_view[:, j, :],
                rhs_view[:, j, :],
                start=(j == 0),
                stop=(j == NCHUNK - 1),
            )
        out = fresh()
        nc.vector.tensor_copy(out=out[:], in_=ps[:, 0:128])
        return out

    def mm(lhsT, rhs):
        ps = ps_s.tile([128, 512], F32, tag="small_mm")
        nc.tensor.matmul(ps[:, 0:128], lhsT[:], rhs[:], start=True, stop=True)
        out = fresh()
        nc.vector.tensor_copy(out=out[:], in_=ps[:, 0:128])
        return out

    def add_i(a):
        out = fresh()
        nc.vector.tensor_add(out=out[:], in0=identB[:], in1=a[:])
        return out

    E = accum_mm(D_sb, C_sb)       # E = D^T C
    ET = accum_mm(C_sb, D_sb)      # E^T = C^T D
    E2 = mm(ET, E)
    E2T = mm(E, ET)
    E3 = mm(E2T, E)
    E4 = mm(E2T, E2)
    E4T = mm(E2, E2T)
    E7 = mm(E4T, E3)
    E8 = mm(E4T, E4)
    E8T = mm(E4, E4T)
    E15 = mm(E8T, E7)
    A1T = add_i(ET)
    A2 = add_i(E2)
    A4 = add_i(E4)
    A8 = add_i(E8)
    T1T = mm(A2, A1T)
    T2T = mm(A4, T1T)
    S16 = mm(T2T, A8)
    F_sb = fresh()
    tmp2 = fresh()
    nc.vector.tensor_add(out=tmp2[:], in0=S16[:], in1=S16[:])
    nc.vector.tensor_sub(out=F_sb[:], in0=tmp2[:], in1=E15[:])

    # S = F^T C^T in fp32 PSUM, rounded to fp16 on copy-out
    for nblk in range(4):
        ps = ps_s.tile([128, 512], F32, tag="small_mm")
        nc.tensor.matmul(
            ps[:], F_sb[:], CT[:, nblk * 512 : (nblk + 1) * 512],
            start=True, stop=True,
        )
        nc.scalar.copy(out=S_sb[:, nblk * 512 : (nblk + 1) * 512], in_=ps[:])

    return D_sb


def _emit(nc, tc, ctx):
    x_d = nc.dram_tensor("x", [TOK, N], F16, kind="ExternalInput")
    w_d = nc.dram_tensor("w", [128, 3 * N], F16, kind="ExternalInput")  # [C|D|C^T]
    y_d = nc.dram_tensor("y", [TOK, N], F16, kind="ExternalOutput")

    # main pools FIRST so the setup pool can't alias the streaming tiles
    const = ctx.enter_context(tc.tile_pool(name="const", bufs=1))
    xpool = ctx.enter_context(tc.tile_pool(name="xpool", bufs=NTILE))
    xtpool = ctx.enter_context(tc.tile_pool(name="xtpool", bufs=2))
    ptpool = ctx.enter_context(tc.tile_pool(name="ptpool", bufs=2))
    ypool = ctx.enter_context(tc.tile_pool(name="ypool", bufs=3))
    ps_t = ctx.enter_context(tc.tile_pool(name="ps_t", bufs=2, space="PSUM"))
    ps_p = ctx.enter_context(tc.tile_pool(name="ps_p", bufs=1, space="PSUM"))
    ps_c = ctx.enter_context(tc.tile_pool(name="ps_c", bufs=2, space="PSUM"))
    setup = ctx.enter_context(tc.tile_pool(name="setup", bufs=1))
    ps_s = ctx.enter_context(tc.tile_pool(name="ps_s", bufs=1, space="PSUM"))

    identB = const.tile([128, 128], F16)
    make_identity(nc, identB[:])
    S_sb = const.tile([128, N], F16)

    x_r = x_d[:].rearrange("(t s p) f -> t p s f", p=128, s=NSUB)
    y_r = y_d[:].rearrange("(ts p) f -> ts p f", p=128)

    x_tiles = {}
    xt_tiles = {}
    pt_tiles = {}
    D_view = [None]

    def load(t, split=False):
        x_t = xpool.tile([128, NSUB, N], F16, tag="x_t", name=f"x_t{t}")
        x_tiles[t] = x_t
        if split:
            # first sub-block lands in two half-width pieces so the first
            # transposes (and the PE clock-gate ramp) start ~2us earlier
            nc.sync.dma_start(out=x_t[:, 0, 0:1024], in_=x_r[t][:, 0, 0:1024])
            nc.sync.dma_start(out=x_t[:, 0, 1024:N], in_=x_r[t][:, 0, 1024:N])
            for s in range(1, NSUB):
                nc.sync.dma_start(out=x_t[:, s, :], in_=x_r[t][:, s, :])
        else:
            nc.sync.dma_start(out=x_t[:], in_=x_r[t])

    def transpose_tile(t):
        """PE-transpose tile t into xt chunks (ACT copies out of PSUM)."""
        x_t = x_tiles[t]
        xt = xtpool.tile([128, NCHUNK, TTOK], F16, tag="xt")
        xt_tiles[t] = xt
        for i in range(NSUB):
            for g in range(2):
                ps = ps_t.tile([128, 1024], F16, tag="ps_t")
                for jj in range(8):
                    j = g * 8 + jj
                    nc.tensor.transpose(
                        ps[:, jj * 128 : (jj + 1) * 128],
                        x_t[:, i, j * 128 : (j + 1) * 128],
                        identB[:],
                    )
                nc.scalar.copy(
                    out=xt[:, g * 8 : (g + 1) * 8, i * 128 : (i + 1) * 128],
                    in_=ps[:].rearrange("p (c q) -> p c q", c=8),
                )

    def stage1(t):
        """P^T = D^T x^T for tile t -> pt."""
        xt = xt_tiles[t]
        D_sb = D_view[0]
        psp = ps_p.tile([128, TTOK], F32, tag="ps_p")
        for j in range(NCHUNK):
            nc.tensor.matmul(
                psp[:],
                D_sb[:, j, :],
                xt[:, j, :],
                start=(j == 0),
                stop=(j == NCHUNK - 1),
            )
        pt = ptpool.tile([128, TTOK], F16, tag="pt")
        nc.vector.tensor_copy(out=pt[:], in_=psp[:])
        pt_tiles[t] = pt

    def head(t):
        transpose_tile(t)
        stage1(t)

    def tail(t):
        """stage2 + add + store for tile t."""
        x_t = x_tiles[t]
        pt = pt_tiles[t]
        for i in range(NSUB):
            y_h = ypool.tile([128, N], F16, tag="y_h")
            for nb2 in range(2):
                psc = ps_c.tile([128, 1024], F32, tag="ps_c")
                for k in range(2):
                    nblk = nb2 * 2 + k
                    nc.tensor.matmul(
                        psc[:, k * 512 : (k + 1) * 512],
                        pt[:, i * 128 : (i + 1) * 128],
                        S_sb[:, nblk * 512 : (nblk + 1) * 512],
                        start=True,
                        stop=True,
                    )
                nc.vector.tensor_add(
                    out=y_h[:, nb2 * 1024 : (nb2 + 1) * 1024],
                    in0=psc[:],
                    in1=x_t[:, i, nb2 * 1024 : (nb2 + 1) * 1024],
                )
            nc.sync.dma_start(out=y_r[t * NSUB + i], in_=y_h[:])

    # ---- emission order == engine queue order ----
    # HAM primer: ~2.7us of dummy PE transposes right after the preamble
    # (the PE is otherwise idle until x data lands at ~9.8us) so the PE
    # clock-gate's activity window flips to 8/8 several us earlier.  Eight
    # distinct 128-col regions per rotating tile -> no same-region WAW sems.
    for _pr in range(6):
        ps_pr = ps_c.tile([128, 1024], F16, tag="ps_c", name=f"prime{_pr}")
        for _k in range(8):
            nc.tensor.transpose(
                ps_pr[:, _k * 128 : (_k + 1) * 128], identB[:], identB[:]
            )
    load(0, split=True)
    for t in range(1, NTILE):
        load(t)
    transpose_tile(0)                       # PE queue starts on transposes
    D_view[0] = _chain(nc, tc, w_d, const, setup, ps_s, identB, S_sb)
    stage1(0)
    for t in range(1, NTILE):
        tail(t - 1)
        head(t)
    tail(NTILE - 1)


def build_nc():
    key = ("v17",)
    if key in _NC_CACHE:
        return _NC_CACHE[key]
    nc = bacc.Bacc(
        "TRN2",
        target_bir_lowering=False,
        debug=False,
        enable_asserts=False,
        num_devices=NCORES,
    )
    from contextlib import ExitStack

    with tile.TileContext(nc) as tc, ExitStack() as ctx:
        _emit(nc, tc, ctx)
    nc.compile()
    _NC_CACHE[key] = nc
    return nc


def _host_weights(U, V):
    """Marshal U, V into the device layout (concat/negate/transpose only)."""
    U = np.ascontiguousarray(U, dtype=np.float32)
    V = np.ascontiguousarray(V, dtype=np.float32)
    C = np.concatenate([U, V], axis=1)                     # [n, 2r]
    D = np.concatenate([V, -U], axis=1)                    # [n, 2r]
    # partition-major: row p holds chunks j -> C[j*128+p, :]
    Cp = np.ascontiguousarray(
        C.reshape(NCHUNK, 128, R2).transpose(1, 0, 2).reshape(128, N)
    ).astype(HDT)
    Dp = np.ascontiguousarray(
        D.reshape(NCHUNK, 128, R2).transpose(1, 0, 2).reshape(128, N)
    ).astype(HDT)
    CTh = np.ascontiguousarray(C.T).astype(HDT)            # [2r, n]
    return np.ascontiguousarray(np.concatenate([Cp, Dp, CTh], axis=1))


def _run(input, U, V, trace=False, tmpdir=None, **bkw):
    nc = build_nc()
    W16 = _host_weights(U, V)
    in_maps = [
        {"x": np.ascontiguousarray(input[c]).astype(HDT), "w": W16}
        for c in range(NCORES)
    ]
    res = run_bass_kernel_spmd(
        nc, in_maps, list(range(NCORES)), trace=trace, tmpdir=tmpdir, **bkw
    )
    out = np.stack(
        [res.results[c]["y"].astype(np.float32) for c in range(NCORES)], axis=0
    )
    return out, res


def kernel(input, U, V):
    out, _ = _run(input, U, V, trace=False)
    return out


# revision 17
# speedup vs baseline: 1.1699x; 1.1699x over previous
"""Low-rank Cayley linear kernel for TRN2 (8 NeuronCores, batch-sharded).

Math: reference computes W = (I+A) @ NS4(I-A) with A = U V^T - V U^T and
NS4 = 4 Newton-Schulz iterations for (I-A)^{-1} starting at X=I, which is
exactly the partial Neumann sum X4 = sum_{j=0}^{15} A^j.  With
C = [U, V] (n x 2r), D = [V, -U] (n x 2r) we have A = C D^T and
A^{j+1} = C E^j D^T where E = D^T C is (2r x 2r).  Therefore

    W = (I + A) X4 = I + C F D^T,   F = 2 * sum_{j=0}^{14} E^j + E^15

and the output is

    y = x @ W^T = x + (x @ D) @ (F^T C^T).

All the 2048^3 work collapses to two rank-128 GEMMs per token plus a
128x128 polynomial evaluated once (8x fewer FLOPs).

v16 (fp16 end to end, measured rel err ~1.2e-3, gate 2e-2):
  - x up / y down as fp16 (8 MB each per core); fp16 transposes + GEMM
    operands at the 1-cycle/row PE rate, fp32 PSUM accumulation.
  - Host marshals [C | D | C^T] into one partition-major fp16 tensor
    (fat descriptors; a row-major layout costs ~14 us of HWDGE
    descriptor generation per load and stalls the in-order PE queue).
  - The whole Newton-Schulz chain runs in fp16 (fp32 PSUM) off the
    uploaded C/D; S = F^T C^T uses the uploaded C^T directly.
  - 256-token tiles: per tile, PE-transpose 128x128 blocks -> x^T
    chunks (ACT copies 1024-wide out of PSUM), stage1 P^T = D^T x^T
    (16 accumulating fp16 matmuls), stage2 corr = P @ S as [128,1024]
    PSUM pairs, y = x + corr on DVE (1024-wide) writing fp16, 0.5 MB
    stores right behind each sub-block's add.
  - Emission order = engine queue order: x loads and the first tile's
    transposes are emitted BEFORE the chain so the in-order PE queue
    starts on transposes (x data lands ~8.5 us) instead of blocking on
    the weight load (~12.5 us).
  - All pools persist; main pools are created first so nothing aliases
    the streaming tiles (an alias adds a WAR edge that serializes the
    x loads behind the setup chain).
"""

import numpy as np

import concourse.bacc as bacc
import concourse.bass as bass
import concourse.mybir as mybir
import concourse.tile as tile
from concourse.bass_utils import run_bass_kernel_spmd
from concourse.masks import make_identity

N = 2048          # model dim (N_IN == N_OUT)
R = 64            # rank of U, V
R2 = 2 * R        # 128
NCORES = 8
TOK = 2048        # tokens per core (one batch element)
F32 = mybir.dt.float32
F16 = mybir.dt.float16
NCHUNK = N // 128          # 16 feature chunks
NTILE = 8                  # 256-token tiles
NSUB = 2                   # 128-token sub-blocks per tile
TTOK = NSUB * 128          # tokens per tile
HDT = np.float16

_NC_CACHE = {}


def _chain(nc, tc, w_d, const, setup, ps_s, identB, S_sb):
    """Emit weight construction (E powers -> F -> S).  Returns D_sb view."""
    # fused weight tile: [C | D | C^T] on the scalar HWDGE ring; C|D land
    # first (they gate the E accumulation), C^T arrives later
    wsb = const.tile([128, 3, N], F16)
    w_r = w_d[:].rearrange("p (t f) -> p t f", t=3)
    nc.scalar.dma_start(out=wsb[:, 0:2], in_=w_r[:, 0:2])
    nc.scalar.dma_start(out=wsb[:, 2], in_=w_r[:, 2])
    C_sb = wsb[:, 0].rearrange("p (j q) -> p j q", j=NCHUNK)
    D_sb = wsb[:, 1].rearrange("p (j q) -> p j q", j=NCHUNK)
    CT = wsb[:, 2]

    counter = [0]

    def fresh():
        counter[0] += 1
        return setup.tile([128, 128], F16, name=f"sm{counter[0]}", tag=f"sm{counter[0]}")

    def accum_mm(lhs_view, rhs_view):
        ps = ps_s.tile([128, 512], F32, tag="small_mm")
        for j in range(NCHUNK):
            nc.tensor.matmul(
                ps[:, 0:128],
                lhs_view[:, j, :],
                rhs_view[:, j, :],
                start=(j == 0),
                stop=(j == NCHUNK - 1),
            )
        out = fresh()
        nc.vector.tensor_copy(out=out[:], in_=ps[:, 0:128])
        return out

    def mm(lhsT, rhs):
        ps = ps_s.tile([128, 512], F32, tag="small_mm")
        nc.tensor.matmul(ps[:, 0:128], lhsT[:], rhs[:], start=True, stop=True)
        out = fresh()
        nc.vector.tensor_copy(out=out[:], in_=ps[:, 0:128])
        return out

    def add_i(a):
        out = fresh()
        nc.vector.tensor_add(out=out[:], in0=identB[:], in1=a[:])
        return out

    E = accum_mm(D_sb, C_sb)       # E = D^T C
    ET = accum_mm(C_sb, D_sb)      # E^T = C^T D
    E2 = mm(ET, E)
    E2T = mm(E, ET)
    E3 = mm(E2T, E)
    E4 = mm(E2T, E2)
    E4T = mm(E2, E2T)
    E7 = mm(E4T, E3)
    E8 = mm(E4T, E4)
    E8T = mm(E4, E4T)
    E15 = mm(E8T, E7)
    A1T = add_i(ET)
    A2 = add_i(E2)
    A4 = add_i(E4)
    A8 = add_i(E8)
    T1T = mm(A2, A1T)
    T2T = mm(A4, T1T)
    S16 = mm(T2T, A8)
    F_sb = fresh()
    tmp2 = fresh()
    nc.vector.tensor_add(out=tmp2[:], in0=S16[:], in1=S16[:])
    nc.vector.tensor_sub(out=F_sb[:], in0=tmp2[:], in1=E15[:])

    # S = F^T C^T in fp32 PSUM, rounded to fp16 on copy-out
    for nblk in range(4):
        ps = ps_s.tile([128, 512], F32, tag="small_mm")
        nc.tensor.matmul(
            ps[:], F_sb[:], CT[:, nblk * 512 : (nblk + 1) * 512],
            start=True, stop=True,
        )
        nc.scalar.copy(out=S_sb[:, nblk * 512 : (nblk + 1) * 512], in_=ps[:])

    return D_sb


def _emit(nc, tc, ctx):
    x_d = nc.dram_tensor("x", [TOK, N], F16, kind="ExternalInput")
    w_d = nc.dram_tensor("w", [128, 3 * N], F16, kind="ExternalInput")  # [C|D|C^T]
    y_d = nc.dram_tensor("y", [TOK, N], F16, kind="ExternalOutput")

    # main pools FIRST so the setup pool can't alias the streaming tiles
    const = ctx.enter_context(tc.tile_pool(name="const", bufs=1))
    xpool = ctx.enter_context(tc.tile_pool(name="xpool", bufs=NTILE))
    xtpool = ctx.enter_context(tc.tile_pool(name="xtpool", bufs=2))
    ptpool = ctx.enter_context(tc.tile_pool(name="ptpool", bufs=2))
    ypool = ctx.enter_context(tc.tile_pool(name="ypool", bufs=3))
    ps_t = ctx.enter_context(tc.tile_pool(name="ps_t", bufs=2, space="PSUM"))
    ps_p = ctx.enter_context(tc.tile_pool(name="ps_p", bufs=1, space="PSUM"))
    ps_c = ctx.enter_context(tc.tile_pool(name="ps_c", bufs=2, space="PSUM"))
    setup = ctx.enter_context(tc.tile_pool(name="setup", bufs=1))
    ps_s = ctx.enter_context(tc.tile_pool(name="ps_s", bufs=1, space="PSUM"))

    identB = const.tile([128, 128], F16)
    make_identity(nc, identB[:])
    S_sb = const.tile([128, N], F16)

    x_r = x_d[:].rearrange("(t s p) f -> t p s f", p=128, s=NSUB)
    y_r = y_d[:].rearrange("(ts p) f -> ts p f", p=128)

    x_tiles = {}
    xt_tiles = {}
    pt_tiles = {}
    D_view = [None]

    def load(t, split=False):
        x_t = xpool.tile([128, NSUB, N], F16, tag="x_t", name=f"x_t{t}")
        x_tiles[t] = x_t
        if split:
            # first sub-block lands in two half-width pieces so the first
            # transposes (and the PE clock-gate ramp) start ~2us earlier
            nc.sync.dma_start(out=x_t[:, 0, 0:1024], in_=x_r[t][:, 0, 0:1024])
            nc.sync.dma_start(out=x_t[:, 0, 1024:N], in_=x_r[t][:, 0, 1024:N])
            for s in range(1, NSUB):
                nc.sync.dma_start(out=x_t[:, s, :], in_=x_r[t][:, s, :])
        else:
            nc.sync.dma_start(out=x_t[:], in_=x_r[t])

    def transpose_tile(t):
        """PE-transpose tile t into xt chunks (ACT copies out of PSUM)."""
        x_t = x_tiles[t]
        xt = xtpool.tile([128, NCHUNK, TTOK], F16, tag="xt")
        xt_tiles[t] = xt
        for i in range(NSUB):
            for g in range(2):
                ps = ps_t.tile([128, 1024], F16, tag="ps_t")
                for jj in range(8):
                    j = g * 8 + jj
                    nc.tensor.transpose(
                        ps[:, jj * 128 : (jj + 1) * 128],
                        x_t[:, i, j * 128 : (j + 1) * 128],
                        identB[:],
                    )
                nc.scalar.copy(
                    out=xt[:, g * 8 : (g + 1) * 8, i * 128 : (i + 1) * 128],
                    in_=ps[:].rearrange("p (c q) -> p c q", c=8),
                )

    def stage1(t):
        """P^T = D^T x^T for tile t -> pt."""
        xt = xt_tiles[t]
        D_sb = D_view[0]
        psp = ps_p.tile([128, TTOK], F32, tag="ps_p")
        for j in range(NCHUNK):
            nc.tensor.matmul(
                psp[:],
                D_sb[:, j, :],
                xt[:, j, :],
                start=(j == 0),
                stop=(j == NCHUNK - 1),
            )
        pt = ptpool.tile([128, TTOK], F16, tag="pt")
        nc.scalar.copy(out=pt[:], in_=psp[:])
        pt_tiles[t] = pt

    def head(t):
        transpose_tile(t)
        stage1(t)

    def tail(t):
        """stage2 + add + store for tile t."""
        x_t = x_tiles[t]
        pt = pt_tiles[t]
        for i in range(NSUB):
            y_h = ypool.tile([128, N], F16, tag="y_h")
            for nb2 in range(2):
                psc = ps_c.tile([128, 1024], F32, tag="ps_c")
                for k in range(2):
                    nblk = nb2 * 2 + k
                    nc.tensor.matmul(
                        psc[:, k * 512 : (k + 1) * 512],
                        pt[:, i * 128 : (i + 1) * 128],
                        S_sb[:, nblk * 512 : (nblk + 1) * 512],
                        start=True,
                        stop=True,
                    )
                nc.vector.tensor_add(
                    out=y_h[:, nb2 * 1024 : (nb2 + 1) * 1024],
                    in0=psc[:],
                    in1=x_t[:, i, nb2 * 1024 : (nb2 + 1) * 1024],
                )
            nc.sync.dma_start(out=y_r[t * NSUB + i], in_=y_h[:])

    # ---- emission order == engine queue order ----
    load(0, split=True)
    for t in range(1, NTILE):
        load(t)
    transpose_tile(0)                       # PE queue starts on transposes
    D_view[0] = _chain(nc, tc, w_d, const, setup, ps_s, identB, S_sb)
    stage1(0)
    for t in range(1, NTILE):
        tail(t - 1)
        head(t)
    tail(NTILE - 1)


def build_nc():
    key = ("v16",)
    if key in _NC_CACHE:
        return _NC_CACHE[key]
    nc = bacc.Bacc(
        "TRN2",
        target_bir_lowering=False,
        debug=False,
        enable_asserts=False,
        num_devices=NCORES,
    )
    from contextlib import ExitStack

    with tile.TileContext(nc) as tc, ExitStack() as ctx:
        _emit(nc, tc, ctx)
    nc.compile()
    _NC_CACHE[key] = nc
    return nc


def _host_weights(U, V):
    """Marshal U, V into the device layout (concat/negate/transpose only)."""
    U = np.ascontiguousarray(U, dtype=np.float32)
    V = np.ascontiguousarray(V, dtype=np.float32)
    C = np.concatenate([U, V], axis=1)                     # [n, 2r]
    D = np.concatenate([V, -U], axis=1)                    # [n, 2r]
    # partition-major: row p holds chunks j -> C[j*128+p, :]
    Cp = np.ascontiguousarray(
        C.reshape(NCHUNK, 128, R2).transpose(1, 0, 2).reshape(128, N)
    ).astype(HDT)
    Dp = np.ascontiguousarray(
        D.reshape(NCHUNK, 128, R2).transpose(1, 0, 2).reshape(128, N)
    ).astype(HDT)
    CTh = np.ascontiguousarray(C.T).astype(HDT)            # [2r, n]
    return np.ascontiguousarray(np.concatenate([Cp, Dp, CTh], axis=1))


def _run(input, U, V, trace=False, tmpdir=None, **bkw):
    nc = build_nc()
    W16 = _host_weights(U, V)
    in_maps = [
        {"x": np.ascontiguousarray(input[c]).astype(HDT), "w": W16}
        for c in range(NCORES)
    ]
    res = run_bass_kernel_spmd(
        nc, in_maps, list(range(NCORES)), trace=trace, tmpdir=tmpdir, **bkw
    )
    out = np.stack(
        [res.results[c]["y"].astype(np.float32) for c in range(NCORES)], axis=0
    )
    return out, res


def kernel(input, U, V):
    out, _ = _run(input, U, V, trace=False)
    return out
